# revision 5
# baseline (speedup 1.0000x reference)
"""BART attention (B=4, S=2048, D=1024, H=16) on 8 Trainium2 NeuronCores.

Sharding: DP4 x TP2.  Core c owns batch c//2 and head half c%2 (8 heads =
512 projection dims), processed as 4 head-pair "slices" of 128 dims each.
Host sums the two partial y's per batch and adds bo.

Per-core schedule (all matmul inputs bf16; PSUM accumulates f32):
  - x for the core's batch is DMA'd once and stays resident in SBUF.
  - slice s+1's q/k/v projections are interleaved into slice s's attention
    so the PE never stalls behind the softmax-exp stream on ScalarE.
  - v is computed directly in [token, head-dim] orientation (no transposes);
    q/k biases ride the PSUM->SBUF drain (DVE tensor_scalar_add); v bias is
    a rank-1 ones x bv matmul into the same PSUM tile.
  - softmax: exp on ScalarE (fused 1/8 scale); denominators come free as
    extra ones-columns in v_comb -> one PSUM row each; per-q-chunk
    normalization: copy the two sums rows to partition 0, one DVE
    reciprocal, two rank-1 broadcast matmuls, one DVE multiply.  No DMA
    round trips.
  - out-proj runs at the tail (contraction over all 4 slices accumulating
    in PSUM), overlapped with the last slice's attention; y chunks DMA
    straight from PSUM.
"""
import numpy as np
import ml_dtypes

import concourse.bass as bass
import concourse.mybir as mybir
import concourse.tile as tile
from concourse.bass_utils import run_bass_kernel_spmd
from concourse.vector_clock import ScopedClock

F32 = mybir.dt.float32
F32R = mybir.dt.float32r
BF16 = mybir.dt.bfloat16
EXPF = mybir.ActivationFunctionType.Exp

B, S, D = 4, 2048, 1024
NCORES = 8
P = 128                        # partitions / head-dims per slice
DK = 64                        # head dim
KC = D // P                    # 8 contraction chunks for projections
TCH = 512                      # token chunk (projection N / q-chunk)
NCH = S // TCH                 # 4 token chunks per batch
NSL = 4                        # head-pair slices per core (4*128 = 512 dims)
NKT = S // P                   # 16 k-tiles per q-chunk
VW = 2 * P                     # 256: [vA | 1 | 0pad][vB | 1 | 0pad] -- each
                               # half is a 128-wide FWL-eligible lhsT
NM = D // P                    # 8 output-dim chunks of out-proj

# ---------------------------------------------------------------------------
# walrus in this toolchain encodes at most ONE sync wait per instruction
# (two on EventSemaphore).  Tile emits more.  Legalize by carrying excess
# waits on same-engine NOPs inserted right before the instruction (engines
# execute in order, so this is equivalent), and by splitting the kernel-tail
# drain's global-clock waits across a chain of drains.
# ---------------------------------------------------------------------------
_split_counter = [0]


def _legalize_waits(nc):
    inserted = 0
    for fn in nc.m.functions:
        for bb in fn.blocks:
            new_insts = []
            changed = False
            for inst in bb.instructions:
                si = inst.sync_info
                waits = list(si.on_wait) if si is not None and si.on_wait else []
                cap = 2 if inst.opcode == "EventSemaphore" else 1
                if len(waits) > cap:
                    excess, keep = waits[:-cap], waits[-cap:]
                    for w in excess:
                        _split_counter[0] += 1
                        nop = mybir.InstNoOp(
                            name=f"I-waitsplit-{_split_counter[0]}", ins=[], outs=[]
                        )
                        nop.engine = inst.engine
                        nop.sync_info = mybir.SyncInfo(on_wait=[w], on_update=[])
                        new_insts.append(nop)
                        inserted += 1
                    si.on_wait = keep
                    changed = True
                new_insts.append(inst)
            if changed:
                bb.instructions.clear()
                for i in new_insts:
                    bb.instructions.append(i)
    return inserted


class _TC(tile.TileContext):
    def _drain_and_barrier(self, tick_clock, wait_clock):
        drain_inst = self.nc.sync.drain()
        wait_clock.add_sem_waits(
            drain_inst.ins, ScopedClock({None: tick_clock.global_clock})
        )
        si = drain_inst.ins.sync_info
        waits = list(si.on_wait or []) if si is not None else []
        if len(waits) > 1:
            si.on_wait = [waits[0]]
            for w in waits[1:]:
                d = self.nc.sync.drain()
                dsi = d.ins.sync_info
                if dsi is None:
                    d.ins.sync_info = mybir.SyncInfo(on_wait=[w], on_update=[])
                else:
                    dsi.on_wait = [w]
        self.nc.all_engine_barrier()
        assert self.sems is not None
        popped = self.nc._tile_sem_poison_stack.pop()
        assert popped is self._sem_poison
        self.nc.clear_and_free_semaphores(list(self.sems.allocated().values()))
        self.nc.all_engine_barrier()


# ---------------------------------------------------------------------------
# device program (identical on all 8 cores; only input data differs)
# ---------------------------------------------------------------------------
def _build_nc(repeat=1):
    nc = bass.Bass("TRN2", target_bir_lowering=False, debug=False,
                   num_devices=NCORES)
    xt = nc.dram_tensor("xt", [D, S], BF16, kind="ExternalInput").ap()
    wqm = nc.dram_tensor("wqm", [D, NSL * P], BF16, kind="ExternalInput").ap()
    wkm = nc.dram_tensor("wkm", [D, NSL * P], BF16, kind="ExternalInput").ap()
    wvm = nc.dram_tensor("wvm", [D, NSL * P], BF16, kind="ExternalInput").ap()
    wqb = nc.dram_tensor("wqb", [NSL, P], F32, kind="ExternalInput").ap()
    wkb = nc.dram_tensor("wkb", [NSL, P], F32, kind="ExternalInput").ap()
    wot = nc.dram_tensor("wo", [NSL * P, D], BF16, kind="ExternalInput").ap()
    yt = nc.dram_tensor("yt", [D, S], BF16, kind="ExternalOutput").ap()

    with _TC(nc) as tc, nc.allow_low_precision(
            reason="bf16 matmul inputs; 2e-2 harness tolerance"):
        _emit(nc, tc, xt, wqm, wkm, wvm, wqb, wkb, wot, yt, repeat=repeat)
    _legalize_waits(nc)
    return nc


def _emit(nc, tc, xt, wqm, wkm, wvm, wqb, wkb, wot, yt, repeat=1):
    ctxs = []

    def pool(name, bufs, space="SBUF"):
        p = tc.tile_pool(name=name, bufs=bufs, space=space)
        ctxs.append(p)
        return p.__enter__()

    wpool = pool("w", 1)
    persist = pool("persist", 1)
    qkpool = pool("qk", 2)
    vpool = pool("v", 2)
    epool = pool("e", 4)
    sumpool = pool("sums", 2)
    ypool = pool("yst", 2)
    spool = pool("ps_s", 2, space="PSUM")      # [128,1024] f32 = 2 banks/slot
    opool = pool("ps_o", 2, space="PSUM")      # [65,512] 1 bank/slot (A+B)
    gpool = pool("ps_g", 2, space="PSUM")      # [128,512] 1 bank/slot (shared)

    # ---- constants / weights / resident x ----
    # DMA queue order is issue order: x chunk 0 and wq first so the first
    # projection matmuls start as early as possible.
    x_res = persist.tile([P, KC, S], BF16)      # resident x [d%128, d//128, t]
    wq_sb = wpool.tile([P, KC, NSL * P], BF16)
    wk_sb = wpool.tile([P, KC, NSL * P], BF16)
    wv_sb = wpool.tile([P, KC, NSL * P], BF16)
    wo_sb = wpool.tile([P, NSL, D], BF16)
    bq_sb = wpool.tile([P, NSL], F32)
    bk_sb = wpool.tile([P, NSL], F32)

    def xload(c0):
        nc.sync.dma_start(
            x_res[:, :, c0:c0 + TCH],
            xt[:, c0:c0 + TCH].rearrange("(k p) n -> p k n", p=P))

    xload(0)
    wqr = wqm.rearrange("(k p) d -> p k d", p=P)
    wkr = wkm.rearrange("(k p) d -> p k d", p=P)
    wvr = wvm.rearrange("(k p) d -> p k d", p=P)
    nc.sync.dma_start(wq_sb[:, :, 0:P], wqr[:, :, 0:P])
    nc.sync.dma_start(bq_sb[:], wqb.rearrange("s p -> p s"))
    nc.sync.dma_start(wk_sb[:, :, 0:P], wkr[:, :, 0:P])
    nc.sync.dma_start(bk_sb[:], wkb.rearrange("s p -> p s"))
    nc.sync.dma_start(wv_sb[:, :, 0:P], wvr[:, :, 0:P])
    for c in range(1, NCH):
        xload(c * TCH)
    nc.sync.dma_start(wq_sb[:, :, P:NSL * P], wqr[:, :, P:NSL * P])
    nc.sync.dma_start(wk_sb[:, :, P:NSL * P], wkr[:, :, P:NSL * P])
    nc.sync.dma_start(wv_sb[:, :, P:NSL * P], wvr[:, :, P:NSL * P])
    nc.sync.dma_start(wo_sb[:], wot.rearrange("(s p) d -> p s d", p=P))

    ones_f32 = wpool.tile([P, TCH], F32)
    nc.vector.memset(ones_f32[:], 1.0)
    ones_bf = wpool.tile([1, TCH], BF16)
    nc.vector.tensor_copy(ones_bf[:], ones_f32[0:1, :])
    ones_r = wpool.tile([1, DK], F32R)
    nc.vector.tensor_copy(ones_r[:], ones_f32[0:1, 0:DK])

    # all slices' normalized attention outputs: [dim%128, slice, tok]
    oraw = persist.tile([P, NSL, S], BF16)

    # v_comb ping-pong pair is persistent: the zero pad + ones columns are
    # written once here, outside the repeat loop; vdrain only overwrites the
    # vA/vB column blocks.
    v_combs = []
    for pp in range(2):
        v_c = persist.tile([P, NKT, VW], BF16, name=f"v_comb{pp}")
        nc.vector.memset(v_c[:], 0.0)
        for half in range(2):
            o = half * P + DK
            nc.vector.tensor_copy(
                v_c[:, :, o:o + 1], ones_f32[:, 0:1].broadcast_to([P, NKT, 1]))
        v_combs.append(v_c)

    def alloc_slice_tiles(i):
        qT = qkpool.tile([P, S], BF16, tag="qT")
        kT = qkpool.tile([P, S], BF16, tag="kT")
        return qT, kT, v_combs[i % 2]

    # ---------------- projection steps for slice s (generator) -------------
    def proj_steps(s, tiles):
        """Yields closures; each emits a small group of instructions that
        computes slice s's qT/kT/v_comb into `tiles`."""
        qT, kT, v_comb = tiles
        ps = [None]
        for c in range(NCH):
            c0 = c * TCH

            def qkmm(c0, w_sb, lo):
                if lo == 0:
                    ps[0] = gpool.tile([P, TCH], F32, tag="g", name="qk_ps")
                for kc in range(lo, lo + 4):
                    nc.tensor.matmul(ps[0][:], w_sb[:, kc, s * P:(s + 1) * P],
                                     x_res[:, kc, c0:c0 + TCH],
                                     start=(kc == 0), stop=(kc == KC - 1))

            def qkdrain(c0, dst, b_sb):
                nc.vector.tensor_scalar_add(dst[:, c0:c0 + TCH], ps[0][:],
                                            b_sb[:, s:s + 1])

            yield lambda c0=c0: qkmm(c0, wq_sb, 0)
            yield lambda c0=c0: qkmm(c0, wq_sb, 4)
            yield lambda c0=c0: qkdrain(c0, qT, bq_sb)
            yield lambda c0=c0: qkmm(c0, wk_sb, 0)
            yield lambda c0=c0: qkmm(c0, wk_sb, 4)
            yield lambda c0=c0: qkdrain(c0, kT, bk_sb)

            # v in [token, head-dim] orientation: 4 token-tiles of 128.
            # bv is NOT added here: normalized attnv output with biased v is
            # (out + bv), and bv's contribution to y is the constant bv @ wo,
            # which the host adds exactly.
            def vmm(c0, pair):
                if pair == 0:
                    ps[0] = gpool.tile([P, TCH], F32, tag="g", name="v_ps")
                for tt in range(2 * pair, 2 * pair + 2):
                    t0 = c0 + tt * P
                    for kc in range(KC):
                        nc.tensor.matmul(ps[0][:, tt * P:(tt + 1) * P],
                                         x_res[:, kc, t0:t0 + P],
                                         wv_sb[:, kc, s * P:(s + 1) * P],
                                         start=(kc == 0), stop=(kc == KC - 1))

            def vdrain(c0, half):
                vt0 = c0 // P
                o = half * P
                nc.vector.tensor_copy(
                    v_comb[:, vt0:vt0 + 4, o:o + DK],
                    ps[0].rearrange("p (t d) -> p t d", t=4)[:, :,
                                                            half * DK:
                                                            (half + 1) * DK])

            yield lambda c0=c0: vmm(c0, 0)
            yield lambda c0=c0: vmm(c0, 1)
            yield lambda c0=c0: vdrain(c0, 0)
            yield lambda c0=c0: vdrain(c0, 1)

    # ---------------- output projection for q-chunk qc ---------------------
    def outproj(qc):
        q0 = qc * TCH
        for m in range(NM):
            ps_y = gpool.tile([P, TCH], F32, tag="g")
            for s in range(NSL):
                nc.tensor.matmul(ps_y[:], wo_sb[:, s, m * P:(m + 1) * P],
                                 oraw[:, s, q0:q0 + TCH],
                                 start=(s == 0), stop=(s == NSL - 1))
            ys = ypool.tile([P, TCH], BF16, tag="ys")
            nc.vector.tensor_copy(ys[:], ps_y[:])
            nc.sync.dma_start(yt[m * P:(m + 1) * P, q0:q0 + TCH], ys[:])

    # ---------------- attention for slice s, interleaved -------------------
    # sched: global iteration counter + deferred PE-side closures.  The
    # normalization's PE work (bc broadcasts, muls, out-proj) is deferred a
    # few iterations into the NEXT q-chunk so the in-order PE stream never
    # parks behind the DVE reciprocal chain.
    sched = {"it": 0, "defer": []}

    def tick_defer():
        while sched["defer"] and sched["defer"][0][0] <= sched["it"]:
            sched["defer"].pop(0)[1]()

    def attention(s, tiles, inter, outproj_here):
        qT, kT, v_comb = tiles
        pend = []
        for qc in range(NCH):
            q0 = qc * TCH
            ps_oA = opool.tile([P, TCH], F32, tag="o", name="ps_oA")
            ps_oB = opool.tile([P, TCH], F32, tag="o", name="ps_oB")

            def attnv(kc, e_t, ps_oA=ps_oA, ps_oB=ps_oB):
                nc.tensor.matmul(ps_oA[:], v_comb[:, kc, 0:P],
                                 e_t[:, 0:TCH],
                                 start=(kc == 0), stop=(kc == NKT - 1))
                nc.tensor.matmul(ps_oB[:], v_comb[:, kc, P:VW],
                                 e_t[:, TCH:2 * TCH],
                                 start=(kc == 0), stop=(kc == NKT - 1))

            for kc in range(NKT):
                kt0 = kc * P
                s_t = spool.tile([P, 2 * TCH], F32, tag="s")
                nc.tensor.matmul(s_t[:, 0:TCH], kT[0:DK, kt0:kt0 + P],
                                 qT[0:DK, q0:q0 + TCH], start=True, stop=True)
                nc.tensor.matmul(s_t[:, TCH:2 * TCH], kT[DK:P, kt0:kt0 + P],
                                 qT[DK:P, q0:q0 + TCH], start=True, stop=True)
                e_t = epool.tile([P, 2 * TCH], BF16, tag="e")
                nc.scalar.activation(e_t[:], s_t[:], EXPF, scale=0.125)
                if len(pend) >= 2:
                    attnv(*pend.pop(0))
                pend.append((kc, e_t))
                sched["it"] += 1
                tick_defer()
                # spread interleaved projection steps evenly over the whole
                # slice so filler work is still available near the q-chunk
                # boundaries
                if inter:
                    steps, done = inter
                    it = qc * NKT + kc
                    want = ((it + 1) * len(steps)) // (NCH * NKT)
                    while inter[1] < min(want, len(steps)):
                        steps[inter[1]]()
                        inter[1] += 1
            while pend:
                attnv(*pend.pop(0))

            # ---- normalization for q-chunk qc ----
            # v_comb halves are [vA | 1 | 0pad] / [vB | 1 | 0pad]: softmax
            # sums land on partition 64 (32-aligned, as DVE PSUM access
            # requires), v outs on partitions 0:64.  The ps_o reads run now
            # (freeing the accumulators); the PE-side tail is deferred.
            nc.vector.tensor_copy(oraw[0:DK, s, q0:q0 + TCH], ps_oA[0:DK, :])
            nc.vector.tensor_copy(oraw[DK:P, s, q0:q0 + TCH], ps_oB[0:DK, :])
            recip_t = sumpool.tile([1, 2 * TCH], F32R, tag="recip")
            nc.vector.reciprocal(recip_t[:, 0:TCH], ps_oA[DK:DK + 1, :])
            nc.vector.reciprocal(recip_t[:, TCH:2 * TCH], ps_oB[DK:DK + 1, :])

            def norm_tail(s=s, q0=q0, recip_t=recip_t):
                bcA = gpool.tile([P, TCH], F32, tag="g", name="bcA")
                nc.tensor.matmul(bcA[0:DK, :], ones_r[:], recip_t[:, 0:TCH],
                                 start=True, stop=True)
                bcB = gpool.tile([P, TCH], F32, tag="g", name="bcB")
                nc.tensor.matmul(bcB[0:DK, :], ones_r[:],
                                 recip_t[:, TCH:2 * TCH],
                                 start=True, stop=True)
                nc.vector.tensor_mul(oraw[0:DK, s, q0:q0 + TCH],
                                     oraw[0:DK, s, q0:q0 + TCH], bcA[0:DK, :])
                nc.vector.tensor_mul(oraw[DK:P, s, q0:q0 + TCH],
                                     oraw[DK:P, s, q0:q0 + TCH], bcB[0:DK, :])

            sched["defer"].append((sched["it"] + 3, norm_tail))
            if outproj_here:
                sched["defer"].append((sched["it"] + 6,
                                       lambda qc=qc: outproj(qc)))

    # ---------------- top-level schedule -----------------------------------
    total = NSL * repeat
    cur = alloc_slice_tiles(0)
    for st in proj_steps(0, cur):
        st()
    for i in range(total):
        s = i % NSL
        inter = []
        nxt = None
        if i + 1 < total:
            nxt = alloc_slice_tiles(i + 1)
            inter = [list(proj_steps((i + 1) % NSL, nxt)), 0]
        attention(s, cur, inter, outproj_here=(s == NSL - 1))
        cur = nxt
    while sched["defer"]:
        sched["defer"].pop(0)[1]()

    for p in reversed(ctxs):
        p.__exit__(None, None, None)


_CACHED = {}


def _get_nc(repeat=1):
    if repeat not in _CACHED:
        _CACHED[repeat] = _build_nc(repeat=repeat)
    return _CACHED[repeat]


def _make_in_maps(x, wq, bq, wk, bk, wv, bv, wo, bo):
    x = np.asarray(x, np.float32)
    wq, bq = np.asarray(wq, np.float32), np.asarray(bq, np.float32)
    wk, bk = np.asarray(wk, np.float32), np.asarray(bk, np.float32)
    wv, bv = np.asarray(wv, np.float32), np.asarray(bv, np.float32)
    wo = np.asarray(wo, np.float32)
    bf = ml_dtypes.bfloat16
    maps = []
    for c in range(NCORES):
        b, h = c // 2, c % 2
        sl = slice(h * NSL * P, (h + 1) * NSL * P)
        maps.append({
            "xt": np.ascontiguousarray(x[b].T).astype(bf),
            "wqm": np.ascontiguousarray(wq[:, sl]).astype(bf),
            "wkm": np.ascontiguousarray(wk[:, sl]).astype(bf),
            "wvm": np.ascontiguousarray(wv[:, sl]).astype(bf),
            "wqb": np.ascontiguousarray(bq[sl]).reshape(NSL, P),
            "wkb": np.ascontiguousarray(bk[sl]).reshape(NSL, P),
            "wo": np.ascontiguousarray(wo[sl, :]).astype(bf),
        })
    return maps


def _gather(results, bo, bv, wo):
    """results: list of 8 dicts with 'yt' [D, S] partial sums.  The device
    skips the v bias; its exact contribution to y is the constant bv @ wo,
    added here along with bo."""
    bias = (np.asarray(bo, np.float64) +
            np.asarray(bv, np.float64) @ np.asarray(wo, np.float64)
            ).astype(np.float32)
    y = np.empty((B, S, D), np.float32)
    for b in range(B):
        yT = results[2 * b]["yt"].astype(np.float32) + \
            results[2 * b + 1]["yt"].astype(np.float32)
        y[b] = yT.T + bias
    return y


def kernel(x, wq, bq, wk, bk, wv, bv, wo, bo):
    nc = _get_nc()
    in_maps = _make_in_maps(x, wq, bq, wk, bk, wv, bv, wo, bo)
    res = run_bass_kernel_spmd(nc, in_maps, core_ids=list(range(NCORES)),
                               trace=False)
    return _gather(res.results, bo, bv, wo)


# revision 6
# speedup vs baseline: 1.1624x; 1.1624x over previous
"""BART attention (B=4, S=2048, D=1024, H=16) on 8 Trainium2 NeuronCores.

Sharding: DP4 x TP2.  Core c owns batch c//2 and head half c%2 (8 heads =
512 projection dims), processed as 4 head-pair "slices" of 128 dims each.
Host sums the two partial y's per batch and adds bo.

Per-core schedule (all matmul inputs bf16; PSUM accumulates f32):
  - x for the core's batch is DMA'd once and stays resident in SBUF.
  - slice s+1's q/k/v projections are interleaved into slice s's attention
    so the PE never stalls behind the softmax-exp stream on ScalarE.
  - v is computed directly in [token, head-dim] orientation (no transposes);
    q/k biases ride the PSUM->SBUF drain (DVE tensor_scalar_add); v bias is
    a rank-1 ones x bv matmul into the same PSUM tile.
  - softmax: exp on ScalarE (fused 1/8 scale); denominators come free as
    extra ones-columns in v_comb -> one PSUM row each; per-q-chunk
    normalization: copy the two sums rows to partition 0, one DVE
    reciprocal, two rank-1 broadcast matmuls, one DVE multiply.  No DMA
    round trips.
  - out-proj runs at the tail (contraction over all 4 slices accumulating
    in PSUM), overlapped with the last slice's attention; y chunks DMA
    straight from PSUM.
"""
import numpy as np
import ml_dtypes

import concourse.bass as bass
import concourse.mybir as mybir
import concourse.tile as tile
from concourse.bass_utils import run_bass_kernel_spmd
from concourse.masks import make_identity
from concourse.vector_clock import ScopedClock

F32 = mybir.dt.float32
F32R = mybir.dt.float32r
BF16 = mybir.dt.bfloat16
EXPF = mybir.ActivationFunctionType.Exp

B, S, D = 4, 2048, 1024
NCORES = 8
P = 128                        # partitions / head-dims per slice
DK = 64                        # head dim
KC = D // P                    # 8 contraction chunks for projections
TCH = 512                      # token chunk (projection N / q-chunk)
NCH = S // TCH                 # 4 token chunks per batch
NSL = 4                        # head-pair slices per core (4*128 = 512 dims)
NKT = S // P                   # 16 k-tiles per q-chunk
VW = 2 * P                     # 256: [vA | 1 | 0pad][vB | 1 | 0pad] -- each
                               # half is a 128-wide FWL-eligible lhsT
NM = D // P                    # 8 output-dim chunks of out-proj

# ---------------------------------------------------------------------------
# walrus in this toolchain encodes at most ONE sync wait per instruction
# (two on EventSemaphore).  Tile emits more.  Legalize by carrying excess
# waits on same-engine NOPs inserted right before the instruction (engines
# execute in order, so this is equivalent), and by splitting the kernel-tail
# drain's global-clock waits across a chain of drains.
# ---------------------------------------------------------------------------
_split_counter = [0]


def _legalize_waits(nc):
    inserted = 0
    for fn in nc.m.functions:
        for bb in fn.blocks:
            new_insts = []
            changed = False
            for inst in bb.instructions:
                si = inst.sync_info
                waits = list(si.on_wait) if si is not None and si.on_wait else []
                cap = 2 if inst.opcode == "EventSemaphore" else 1
                if len(waits) > cap:
                    excess, keep = waits[:-cap], waits[-cap:]
                    for w in excess:
                        _split_counter[0] += 1
                        nop = mybir.InstNoOp(
                            name=f"I-waitsplit-{_split_counter[0]}", ins=[], outs=[]
                        )
                        nop.engine = inst.engine
                        nop.sync_info = mybir.SyncInfo(on_wait=[w], on_update=[])
                        new_insts.append(nop)
                        inserted += 1
                    si.on_wait = keep
                    changed = True
                new_insts.append(inst)
            if changed:
                bb.instructions.clear()
                for i in new_insts:
                    bb.instructions.append(i)
    return inserted


class _TC(tile.TileContext):
    def _drain_and_barrier(self, tick_clock, wait_clock):
        drain_inst = self.nc.sync.drain()
        wait_clock.add_sem_waits(
            drain_inst.ins, ScopedClock({None: tick_clock.global_clock})
        )
        si = drain_inst.ins.sync_info
        waits = list(si.on_wait or []) if si is not None else []
        if len(waits) > 1:
            si.on_wait = [waits[0]]
            for w in waits[1:]:
                d = self.nc.sync.drain()
                dsi = d.ins.sync_info
                if dsi is None:
                    d.ins.sync_info = mybir.SyncInfo(on_wait=[w], on_update=[])
                else:
                    dsi.on_wait = [w]
        self.nc.all_engine_barrier()
        assert self.sems is not None
        popped = self.nc._tile_sem_poison_stack.pop()
        assert popped is self._sem_poison
        self.nc.clear_and_free_semaphores(list(self.sems.allocated().values()))
        self.nc.all_engine_barrier()


# ---------------------------------------------------------------------------
# device program (identical on all 8 cores; only input data differs)
# ---------------------------------------------------------------------------
def _build_nc(repeat=1):
    nc = bass.Bass("TRN2", target_bir_lowering=False, debug=False,
                   num_devices=NCORES)
    xt = nc.dram_tensor("xt", [D, S], BF16, kind="ExternalInput").ap()
    wqm = nc.dram_tensor("wqm", [D, NSL * P], BF16, kind="ExternalInput").ap()
    wkm = nc.dram_tensor("wkm", [D, NSL * P], BF16, kind="ExternalInput").ap()
    wvm = nc.dram_tensor("wvm", [D, NSL * P], BF16, kind="ExternalInput").ap()
    wqb = nc.dram_tensor("wqb", [NSL, P], F32, kind="ExternalInput").ap()
    wkb = nc.dram_tensor("wkb", [NSL, P], F32, kind="ExternalInput").ap()
    wot = nc.dram_tensor("wo", [NSL * P, D], BF16, kind="ExternalInput").ap()
    yt = nc.dram_tensor("yt", [D, S], BF16, kind="ExternalOutput").ap()

    with _TC(nc) as tc, nc.allow_low_precision(
            reason="bf16 matmul inputs; 2e-2 harness tolerance"):
        _emit(nc, tc, xt, wqm, wkm, wvm, wqb, wkb, wot, yt, repeat=repeat)
    _legalize_waits(nc)
    return nc


def _emit(nc, tc, xt, wqm, wkm, wvm, wqb, wkb, wot, yt, repeat=1):
    ctxs = []

    def pool(name, bufs, space="SBUF"):
        p = tc.tile_pool(name=name, bufs=bufs, space=space)
        ctxs.append(p)
        return p.__enter__()

    wpool = pool("w", 1)
    persist = pool("persist", 1)
    qkpool = pool("qk", 2)
    vpool = pool("v", 2)
    epool = pool("e", 4)
    sumpool = pool("sums", 2)
    ypool = pool("yst", 2)
    spool = pool("ps_s", 2, space="PSUM")      # [128,1024] f32 = 2 banks/slot
    opool = pool("ps_o", 2, space="PSUM")      # [65,512] 1 bank/slot (A+B)
    gpool = pool("ps_g", 2, space="PSUM")      # [128,512] 1 bank/slot (shared)

    # ---- constants / weights / resident x ----
    # DMA queue order is issue order: x chunk 0 and wq first so the first
    # projection matmuls start as early as possible.
    x_res = persist.tile([P, KC, S], BF16)      # resident x [d%128, d//128, t]
    wq_sb = wpool.tile([P, KC, NSL * P], BF16)
    wk_sb = wpool.tile([P, KC, NSL * P], BF16)
    wv_sb = wpool.tile([P, KC, NSL * P], BF16)
    wo_sb = wpool.tile([P, NSL, D], BF16)
    bq_sb = wpool.tile([P, NSL], F32)
    bk_sb = wpool.tile([P, NSL], F32)

    def xload(c0):
        nc.sync.dma_start(
            x_res[:, :, c0:c0 + TCH],
            xt[:, c0:c0 + TCH].rearrange("(k p) n -> p k n", p=P))

    xload(0)
    wqr = wqm.rearrange("(k p) d -> p k d", p=P)
    wkr = wkm.rearrange("(k p) d -> p k d", p=P)
    wvr = wvm.rearrange("(k p) d -> p k d", p=P)
    nc.sync.dma_start(wq_sb[:, :, 0:P], wqr[:, :, 0:P])
    nc.sync.dma_start(bq_sb[:], wqb.rearrange("s p -> p s"))
    nc.sync.dma_start(wk_sb[:, :, 0:P], wkr[:, :, 0:P])
    nc.sync.dma_start(bk_sb[:], wkb.rearrange("s p -> p s"))
    nc.sync.dma_start(wv_sb[:, :, 0:P], wvr[:, :, 0:P])
    for c in range(1, NCH):
        xload(c * TCH)
    nc.sync.dma_start(wq_sb[:, :, P:NSL * P], wqr[:, :, P:NSL * P])
    nc.sync.dma_start(wk_sb[:, :, P:NSL * P], wkr[:, :, P:NSL * P])
    nc.sync.dma_start(wv_sb[:, :, P:NSL * P], wvr[:, :, P:NSL * P])
    nc.sync.dma_start(wo_sb[:], wot.rearrange("(s p) d -> p s d", p=P))

    ones_f32 = wpool.tile([P, TCH], F32)
    nc.vector.memset(ones_f32[:], 1.0)
    ones_r = wpool.tile([1, DK], F32R)
    nc.vector.tensor_copy(ones_r[:], ones_f32[0:1, 0:DK])
    ident_f32 = wpool.tile([P, P], F32)
    make_identity(nc, ident_f32[:])
    ident_bf = wpool.tile([P, P], BF16)
    nc.vector.tensor_copy(ident_bf[:], ident_f32[:])

    # all slices' normalized attention outputs: [dim%128, slice, tok]
    oraw = persist.tile([P, NSL, S], BF16)

    # v_comb ping-pong pair is persistent: the zero pad + ones columns are
    # written once here, outside the repeat loop; vdrain only overwrites the
    # vA/vB column blocks.
    v_combs = []
    for pp in range(2):
        v_c = persist.tile([P, NKT, VW], BF16, name=f"v_comb{pp}")
        nc.vector.memset(v_c[:], 0.0)
        for half in range(2):
            o = half * P + DK
            nc.vector.tensor_copy(
                v_c[:, :, o:o + 1], ones_f32[:, 0:1].broadcast_to([P, NKT, 1]))
        v_combs.append(v_c)

    def alloc_slice_tiles(i):
        qT = qkpool.tile([P, S], BF16, tag="qT")
        kT = qkpool.tile([P, S], BF16, tag="kT")
        return qT, kT, v_combs[i % 2]

    # ---------------- projection steps for slice s (generator) -------------
    def proj_steps(s, tiles):
        """Yields closures; each emits a small group of instructions that
        computes slice s's qT/kT/v_comb into `tiles`."""
        qT, kT, v_comb = tiles
        ps = [None]
        for c in range(NCH):
            c0 = c * TCH

            def qkmm(c0, w_sb, lo):
                if lo == 0:
                    ps[0] = gpool.tile([P, TCH], F32, tag="g", name="qk_ps")
                for kc in range(lo, lo + 4):
                    nc.tensor.matmul(ps[0][:], w_sb[:, kc, s * P:(s + 1) * P],
                                     x_res[:, kc, c0:c0 + TCH],
                                     start=(kc == 0), stop=(kc == KC - 1))

            def qkdrain(c0, dst, b_sb):
                nc.vector.tensor_scalar_add(dst[:, c0:c0 + TCH], ps[0][:],
                                            b_sb[:, s:s + 1])

            yield lambda c0=c0: qkmm(c0, wq_sb, 0)
            yield lambda c0=c0: qkmm(c0, wq_sb, 4)
            yield lambda c0=c0: qkdrain(c0, qT, bq_sb)
            yield lambda c0=c0: qkmm(c0, wk_sb, 0)
            yield lambda c0=c0: qkmm(c0, wk_sb, 4)
            yield lambda c0=c0: qkdrain(c0, kT, bk_sb)

            # v streamed like q/k ([head-dim, tok], N=512 matmuls — far fewer
            # PE instructions than token-major tiles), then transposed into
            # v_comb via the PE with an identity.  bv is NOT added on device:
            # normalized attnv output with biased v is (out + bv), and bv's
            # contribution to y is the constant bv @ wo, which the host adds
            # exactly.
            vscr = [None]

            def vstream(c0, lo):
                if lo == 0:
                    ps[0] = gpool.tile([P, TCH], F32, tag="g", name="v_ps")
                for kc in range(lo, lo + 4):
                    nc.tensor.matmul(ps[0][:], wv_sb[:, kc, s * P:(s + 1) * P],
                                     x_res[:, kc, c0:c0 + TCH],
                                     start=(kc == 0), stop=(kc == KC - 1))

            def vcopy(c0):
                vscr[0] = vpool.tile([P, TCH], BF16, tag="vscr", name="v_scr")
                nc.vector.tensor_copy(vscr[0][:], ps[0][:])

            def vtr(c0, tt):
                vt = c0 // P + tt
                tr = gpool.tile([P, P], BF16, tag="g", name="v_tr")
                nc.tensor.transpose(tr[:], vscr[0][:, tt * P:(tt + 1) * P],
                                    ident_bf[:])
                nc.vector.tensor_copy(v_comb[:, vt, 0:DK], tr[:, 0:DK])
                nc.vector.tensor_copy(v_comb[:, vt, P:P + DK], tr[:, DK:P])

            yield lambda c0=c0: vstream(c0, 0)
            yield lambda c0=c0: vstream(c0, 4)
            yield lambda c0=c0: vcopy(c0)
            for tt in range(4):
                yield lambda c0=c0, tt=tt: vtr(c0, tt)

    # ---------------- output projection for q-chunk qc ---------------------
    def outproj(qc):
        q0 = qc * TCH
        for m in range(NM):
            ps_y = gpool.tile([P, TCH], F32, tag="g")
            for s in range(NSL):
                nc.tensor.matmul(ps_y[:], wo_sb[:, s, m * P:(m + 1) * P],
                                 oraw[:, s, q0:q0 + TCH],
                                 start=(s == 0), stop=(s == NSL - 1))
            ys = ypool.tile([P, TCH], BF16, tag="ys")
            nc.vector.tensor_copy(ys[:], ps_y[:])
            nc.sync.dma_start(yt[m * P:(m + 1) * P, q0:q0 + TCH], ys[:])

    # ---------------- attention for slice s, interleaved -------------------
    # sched: global iteration counter + deferred PE-side closures.  The
    # normalization's PE work (bc broadcasts, muls, out-proj) is deferred a
    # few iterations into the NEXT q-chunk so the in-order PE stream never
    # parks behind the DVE reciprocal chain.
    sched = {"it": 0, "defer": []}

    def tick_defer():
        while sched["defer"] and sched["defer"][0][0] <= sched["it"]:
            sched["defer"].pop(0)[1]()

    def attention(s, tiles, inter, outproj_here):
        qT, kT, v_comb = tiles
        pend = []
        for qc in range(NCH):
            q0 = qc * TCH
            ps_oA = opool.tile([P, TCH], F32, tag="o", name="ps_oA")
            ps_oB = opool.tile([P, TCH], F32, tag="o", name="ps_oB")

            def attnv(kc, e_t, ps_oA=ps_oA, ps_oB=ps_oB):
                nc.tensor.matmul(ps_oA[:], v_comb[:, kc, 0:P],
                                 e_t[:, 0:TCH],
                                 start=(kc == 0), stop=(kc == NKT - 1))
                nc.tensor.matmul(ps_oB[:], v_comb[:, kc, P:VW],
                                 e_t[:, TCH:2 * TCH],
                                 start=(kc == 0), stop=(kc == NKT - 1))

            for kc in range(NKT):
                kt0 = kc * P
                s_t = spool.tile([P, 2 * TCH], F32, tag="s")
                nc.tensor.matmul(s_t[:, 0:TCH], kT[0:DK, kt0:kt0 + P],
                                 qT[0:DK, q0:q0 + TCH], start=True, stop=True)
                nc.tensor.matmul(s_t[:, TCH:2 * TCH], kT[DK:P, kt0:kt0 + P],
                                 qT[DK:P, q0:q0 + TCH], start=True, stop=True)
                e_t = epool.tile([P, 2 * TCH], BF16, tag="e")
                nc.scalar.activation(e_t[:], s_t[:], EXPF, scale=0.125)
                if len(pend) >= 2:
                    attnv(*pend.pop(0))
                pend.append((kc, e_t))
                sched["it"] += 1
                tick_defer()
                # spread interleaved projection steps evenly over the whole
                # slice so filler work is still available near the q-chunk
                # boundaries
                if inter:
                    steps, done = inter
                    it = qc * NKT + kc
                    want = ((it + 1) * len(steps)) // (NCH * NKT)
                    while inter[1] < min(want, len(steps)):
                        steps[inter[1]]()
                        inter[1] += 1
            while pend:
                attnv(*pend.pop(0))

            # ---- normalization for q-chunk qc ----
            # v_comb halves are [vA | 1 | 0pad] / [vB | 1 | 0pad]: softmax
            # sums land on partition 64 (32-aligned, as DVE PSUM access
            # requires), v outs on partitions 0:64.  The ps_o reads run now
            # (freeing the accumulators); the PE-side tail is deferred.
            nc.vector.tensor_copy(oraw[0:DK, s, q0:q0 + TCH], ps_oA[0:DK, :])
            nc.vector.tensor_copy(oraw[DK:P, s, q0:q0 + TCH], ps_oB[0:DK, :])
            recip_t = sumpool.tile([1, 2 * TCH], F32R, tag="recip")
            nc.vector.reciprocal(recip_t[:, 0:TCH], ps_oA[DK:DK + 1, :])
            nc.vector.reciprocal(recip_t[:, TCH:2 * TCH], ps_oB[DK:DK + 1, :])

            def norm_tail(s=s, q0=q0, recip_t=recip_t):
                bcA = gpool.tile([P, TCH], F32, tag="g", name="bcA")
                nc.tensor.matmul(bcA[0:DK, :], ones_r[:], recip_t[:, 0:TCH],
                                 start=True, stop=True)
                bcB = gpool.tile([P, TCH], F32, tag="g", name="bcB")
                nc.tensor.matmul(bcB[0:DK, :], ones_r[:],
                                 recip_t[:, TCH:2 * TCH],
                                 start=True, stop=True)
                nc.vector.tensor_mul(oraw[0:DK, s, q0:q0 + TCH],
                                     oraw[0:DK, s, q0:q0 + TCH], bcA[0:DK, :])
                nc.vector.tensor_mul(oraw[DK:P, s, q0:q0 + TCH],
                                     oraw[DK:P, s, q0:q0 + TCH], bcB[0:DK, :])

            sched["defer"].append((sched["it"] + 3, norm_tail))
            if outproj_here:
                sched["defer"].append((sched["it"] + 6,
                                       lambda qc=qc: outproj(qc)))

    # ---------------- top-level schedule -----------------------------------
    total = NSL * repeat
    cur = alloc_slice_tiles(0)
    for st in proj_steps(0, cur):
        st()
    for i in range(total):
        s = i % NSL
        inter = []
        nxt = None
        if i + 1 < total:
            nxt = alloc_slice_tiles(i + 1)
            inter = [list(proj_steps((i + 1) % NSL, nxt)), 0]
        attention(s, cur, inter, outproj_here=(s == NSL - 1))
        cur = nxt
    while sched["defer"]:
        sched["defer"].pop(0)[1]()

    for p in reversed(ctxs):
        p.__exit__(None, None, None)


_CACHED = {}


def _get_nc(repeat=1):
    if repeat not in _CACHED:
        _CACHED[repeat] = _build_nc(repeat=repeat)
    return _CACHED[repeat]


def _make_in_maps(x, wq, bq, wk, bk, wv, bv, wo, bo):
    x = np.asarray(x, np.float32)
    wq, bq = np.asarray(wq, np.float32), np.asarray(bq, np.float32)
    wk, bk = np.asarray(wk, np.float32), np.asarray(bk, np.float32)
    wv, bv = np.asarray(wv, np.float32), np.asarray(bv, np.float32)
    wo = np.asarray(wo, np.float32)
    bf = ml_dtypes.bfloat16
    maps = []
    for c in range(NCORES):
        b, h = c // 2, c % 2
        sl = slice(h * NSL * P, (h + 1) * NSL * P)
        maps.append({
            "xt": np.ascontiguousarray(x[b].T).astype(bf),
            "wqm": np.ascontiguousarray(wq[:, sl]).astype(bf),
            "wkm": np.ascontiguousarray(wk[:, sl]).astype(bf),
            "wvm": np.ascontiguousarray(wv[:, sl]).astype(bf),
            "wqb": np.ascontiguousarray(bq[sl]).reshape(NSL, P),
            "wkb": np.ascontiguousarray(bk[sl]).reshape(NSL, P),
            "wo": np.ascontiguousarray(wo[sl, :]).astype(bf),
        })
    return maps


def _gather(results, bo, bv, wo):
    """results: list of 8 dicts with 'yt' [D, S] partial sums.  The device
    skips the v bias; its exact contribution to y is the constant bv @ wo,
    added here along with bo."""
    bias = (np.asarray(bo, np.float64) +
            np.asarray(bv, np.float64) @ np.asarray(wo, np.float64)
            ).astype(np.float32)
    y = np.empty((B, S, D), np.float32)
    for b in range(B):
        yT = results[2 * b]["yt"].astype(np.float32) + \
            results[2 * b + 1]["yt"].astype(np.float32)
        y[b] = yT.T + bias
    return y


def kernel(x, wq, bq, wk, bk, wv, bv, wo, bo):
    nc = _get_nc()
    in_maps = _make_in_maps(x, wq, bq, wk, bk, wv, bv, wo, bo)
    res = run_bass_kernel_spmd(nc, in_maps, core_ids=list(range(NCORES)),
                               trace=False)
    return _gather(res.results, bo, bv, wo)


# revision 7
# speedup vs baseline: 1.2148x; 1.0450x over previous
"""BART attention (B=4, S=2048, D=1024, H=16) on 8 Trainium2 NeuronCores.

Sharding: DP4 x TP2.  Core c owns batch c//2 and head half c%2 (8 heads =
512 projection dims), processed as 4 head-pair "slices" of 128 dims each.
Host sums the two partial y's per batch and adds bo.

Per-core schedule (all matmul inputs bf16; PSUM accumulates f32):
  - x for the core's batch is DMA'd once and stays resident in SBUF.
  - slice s+1's q/k/v projections are interleaved into slice s's attention
    (evenly paced filler steps) so the PE never stalls behind the
    softmax-exp stream on ScalarE; attnv runs 3 iterations behind scores.
  - v streams like q/k then transposes via the PE (fewest PE instructions);
    q/k biases ride the PSUM->SBUF drain (DVE tensor_scalar_add); the v
    bias contribution to y is the constant bv @ wo, added on the host.
  - softmax: exp on ScalarE (fused 1/8 scale); denominators come free as
    ones-columns inside the 128-wide zero-padded (FWL-eligible) v_comb
    stationary operands; per-q-chunk normalization reads the PSUM sum rows
    directly (DVE reciprocal), and its PE work (rank-1 broadcast matmuls +
    multiplies) is deferred into the next q-chunk so the in-order PE
    stream never parks behind the DVE chain.  No DMA round trips.
  - out-proj (contraction over all 4 slices accumulating in PSUM) overlaps
    the last slice's attention; y ships as bf16 partials.
"""
import numpy as np
import ml_dtypes

import concourse.bass as bass
import concourse.mybir as mybir
import concourse.tile as tile
from concourse.bass_utils import run_bass_kernel_spmd
from concourse.masks import make_identity
from concourse.vector_clock import ScopedClock

F32 = mybir.dt.float32
F32R = mybir.dt.float32r
BF16 = mybir.dt.bfloat16
EXPF = mybir.ActivationFunctionType.Exp

B, S, D = 4, 2048, 1024
NCORES = 8
P = 128                        # partitions / head-dims per slice
DK = 64                        # head dim
KC = D // P                    # 8 contraction chunks for projections
TCH = 512                      # token chunk (projection N / q-chunk)
NCH = S // TCH                 # 4 token chunks per batch
NSL = 4                        # head-pair slices per core (4*128 = 512 dims)
NKT = S // P                   # 16 k-tiles per q-chunk
VW = 2 * P                     # 256: [vA | 1 | 0pad][vB | 1 | 0pad] -- each
                               # half is a 128-wide FWL-eligible lhsT
NM = D // P                    # 8 output-dim chunks of out-proj

# ---------------------------------------------------------------------------
# walrus in this toolchain encodes at most ONE sync wait per instruction
# (two on EventSemaphore).  Tile emits more.  Legalize by carrying excess
# waits on same-engine NOPs inserted right before the instruction (engines
# execute in order, so this is equivalent), and by splitting the kernel-tail
# drain's global-clock waits across a chain of drains.
# ---------------------------------------------------------------------------
_split_counter = [0]


def _legalize_waits(nc):
    inserted = 0
    for fn in nc.m.functions:
        for bb in fn.blocks:
            new_insts = []
            changed = False
            for inst in bb.instructions:
                si = inst.sync_info
                waits = list(si.on_wait) if si is not None and si.on_wait else []
                cap = 2 if inst.opcode == "EventSemaphore" else 1
                if len(waits) > cap:
                    excess, keep = waits[:-cap], waits[-cap:]
                    for w in excess:
                        _split_counter[0] += 1
                        nop = mybir.InstNoOp(
                            name=f"I-waitsplit-{_split_counter[0]}", ins=[], outs=[]
                        )
                        nop.engine = inst.engine
                        nop.sync_info = mybir.SyncInfo(on_wait=[w], on_update=[])
                        new_insts.append(nop)
                        inserted += 1
                    si.on_wait = keep
                    changed = True
                new_insts.append(inst)
            if changed:
                bb.instructions.clear()
                for i in new_insts:
                    bb.instructions.append(i)
    return inserted


class _TC(tile.TileContext):
    def _drain_and_barrier(self, tick_clock, wait_clock):
        drain_inst = self.nc.sync.drain()
        wait_clock.add_sem_waits(
            drain_inst.ins, ScopedClock({None: tick_clock.global_clock})
        )
        si = drain_inst.ins.sync_info
        waits = list(si.on_wait or []) if si is not None else []
        if len(waits) > 1:
            si.on_wait = [waits[0]]
            for w in waits[1:]:
                d = self.nc.sync.drain()
                dsi = d.ins.sync_info
                if dsi is None:
                    d.ins.sync_info = mybir.SyncInfo(on_wait=[w], on_update=[])
                else:
                    dsi.on_wait = [w]
        self.nc.all_engine_barrier()
        assert self.sems is not None
        popped = self.nc._tile_sem_poison_stack.pop()
        assert popped is self._sem_poison
        self.nc.clear_and_free_semaphores(list(self.sems.allocated().values()))
        self.nc.all_engine_barrier()


# ---------------------------------------------------------------------------
# device program (identical on all 8 cores; only input data differs)
# ---------------------------------------------------------------------------
def _build_nc(repeat=1):
    nc = bass.Bass("TRN2", target_bir_lowering=False, debug=False,
                   num_devices=NCORES)
    xt = nc.dram_tensor("xt", [D, S], BF16, kind="ExternalInput").ap()
    wqm = nc.dram_tensor("wqm", [D, NSL * P], BF16, kind="ExternalInput").ap()
    wkm = nc.dram_tensor("wkm", [D, NSL * P], BF16, kind="ExternalInput").ap()
    wvm = nc.dram_tensor("wvm", [D, NSL * P], BF16, kind="ExternalInput").ap()
    wqb = nc.dram_tensor("wqb", [NSL, P], F32, kind="ExternalInput").ap()
    wkb = nc.dram_tensor("wkb", [NSL, P], F32, kind="ExternalInput").ap()
    wot = nc.dram_tensor("wo", [NSL * P, D], BF16, kind="ExternalInput").ap()
    yt = nc.dram_tensor("yt", [D, S], BF16, kind="ExternalOutput").ap()

    with _TC(nc) as tc, nc.allow_low_precision(
            reason="bf16 matmul inputs; 2e-2 harness tolerance"):
        _emit(nc, tc, xt, wqm, wkm, wvm, wqb, wkb, wot, yt, repeat=repeat)
    _legalize_waits(nc)
    return nc


def _emit(nc, tc, xt, wqm, wkm, wvm, wqb, wkb, wot, yt, repeat=1):
    ctxs = []

    def pool(name, bufs, space="SBUF"):
        p = tc.tile_pool(name=name, bufs=bufs, space=space)
        ctxs.append(p)
        return p.__enter__()

    wpool = pool("w", 1)
    persist = pool("persist", 1)
    qkpool = pool("qk", 2)
    vpool = pool("v", 2)
    epool = pool("e", 4)
    sumpool = pool("sums", 2)
    ypool = pool("yst", 2)
    spool = pool("ps_s", 2, space="PSUM")      # [128,1024] f32 = 2 banks/slot
    opool = pool("ps_o", 2, space="PSUM")      # [65,512] 1 bank/slot (A+B)
    gpool = pool("ps_g", 2, space="PSUM")      # [128,512] 1 bank/slot (shared)

    # ---- constants / weights / resident x ----
    # DMA queue order is issue order: x chunk 0 and wq first so the first
    # projection matmuls start as early as possible.
    x_res = persist.tile([P, KC, S], BF16)      # resident x [d%128, d//128, t]
    wq_sb = wpool.tile([P, KC, NSL * P], BF16)
    wk_sb = wpool.tile([P, KC, NSL * P], BF16)
    wv_sb = wpool.tile([P, KC, NSL * P], BF16)
    wo_sb = wpool.tile([P, NSL, D], BF16)
    bq_sb = wpool.tile([P, NSL], F32)
    bk_sb = wpool.tile([P, NSL], F32)

    def xload(c0):
        nc.sync.dma_start(
            x_res[:, :, c0:c0 + TCH],
            xt[:, c0:c0 + TCH].rearrange("(k p) n -> p k n", p=P))

    xload(0)
    wqr = wqm.rearrange("(k p) d -> p k d", p=P)
    wkr = wkm.rearrange("(k p) d -> p k d", p=P)
    wvr = wvm.rearrange("(k p) d -> p k d", p=P)
    nc.sync.dma_start(wq_sb[:, :, 0:P], wqr[:, :, 0:P])
    nc.sync.dma_start(bq_sb[:], wqb.rearrange("s p -> p s"))
    nc.sync.dma_start(wk_sb[:, :, 0:P], wkr[:, :, 0:P])
    nc.sync.dma_start(bk_sb[:], wkb.rearrange("s p -> p s"))
    nc.sync.dma_start(wv_sb[:, :, 0:P], wvr[:, :, 0:P])
    for c in range(1, NCH):
        xload(c * TCH)
    nc.sync.dma_start(wq_sb[:, :, P:NSL * P], wqr[:, :, P:NSL * P])
    nc.sync.dma_start(wk_sb[:, :, P:NSL * P], wkr[:, :, P:NSL * P])
    nc.sync.dma_start(wv_sb[:, :, P:NSL * P], wvr[:, :, P:NSL * P])
    nc.sync.dma_start(wo_sb[:], wot.rearrange("(s p) d -> p s d", p=P))

    ones_f32 = wpool.tile([P, TCH], F32)
    nc.vector.memset(ones_f32[:], 1.0)
    ones_r = wpool.tile([1, DK], F32R)
    nc.vector.tensor_copy(ones_r[:], ones_f32[0:1, 0:DK])
    ident_f32 = wpool.tile([P, P], F32)
    make_identity(nc, ident_f32[:])
    ident_bf = wpool.tile([P, P], BF16)
    nc.vector.tensor_copy(ident_bf[:], ident_f32[:])

    # all slices' normalized attention outputs: [dim%128, slice, tok]
    oraw = persist.tile([P, NSL, S], BF16)

    # v_comb ping-pong pair is persistent: the zero pad + ones columns are
    # written once here, outside the repeat loop; vdrain only overwrites the
    # vA/vB column blocks.
    v_combs = []
    for pp in range(2):
        v_c = persist.tile([P, NKT, VW], BF16, name=f"v_comb{pp}")
        nc.vector.memset(v_c[:], 0.0)
        for half in range(2):
            o = half * P + DK
            nc.vector.tensor_copy(
                v_c[:, :, o:o + 1], ones_f32[:, 0:1].broadcast_to([P, NKT, 1]))
        v_combs.append(v_c)

    def alloc_slice_tiles(i):
        qT = qkpool.tile([P, S], BF16, tag="qT")
        kT = qkpool.tile([P, S], BF16, tag="kT")
        return qT, kT, v_combs[i % 2]

    # ---------------- projection steps for slice s (generator) -------------
    def proj_steps(s, tiles):
        """Yields closures; each emits a small group of instructions that
        computes slice s's qT/kT/v_comb into `tiles`."""
        qT, kT, v_comb = tiles
        ps = [None]
        for c in range(NCH):
            c0 = c * TCH

            def qkmm(c0, w_sb, lo):
                if lo == 0:
                    ps[0] = gpool.tile([P, TCH], F32, tag="g", name="qk_ps")
                for kc in range(lo, lo + 4):
                    nc.tensor.matmul(ps[0][:], w_sb[:, kc, s * P:(s + 1) * P],
                                     x_res[:, kc, c0:c0 + TCH],
                                     start=(kc == 0), stop=(kc == KC - 1))

            def qkdrain(c0, dst, b_sb):
                nc.vector.tensor_scalar_add(dst[:, c0:c0 + TCH], ps[0][:],
                                            b_sb[:, s:s + 1])

            yield lambda c0=c0: qkmm(c0, wq_sb, 0)
            yield lambda c0=c0: qkmm(c0, wq_sb, 4)
            yield lambda c0=c0: qkdrain(c0, qT, bq_sb)
            yield lambda c0=c0: qkmm(c0, wk_sb, 0)
            yield lambda c0=c0: qkmm(c0, wk_sb, 4)
            yield lambda c0=c0: qkdrain(c0, kT, bk_sb)

            # v streamed like q/k ([head-dim, tok], N=512 matmuls — far fewer
            # PE instructions than token-major tiles), then transposed into
            # v_comb via the PE with an identity.  bv is NOT added on device:
            # normalized attnv output with biased v is (out + bv), and bv's
            # contribution to y is the constant bv @ wo, which the host adds
            # exactly.
            vscr = [None]

            def vstream(c0, lo):
                if lo == 0:
                    ps[0] = gpool.tile([P, TCH], F32, tag="g", name="v_ps")
                for kc in range(lo, lo + 4):
                    nc.tensor.matmul(ps[0][:], wv_sb[:, kc, s * P:(s + 1) * P],
                                     x_res[:, kc, c0:c0 + TCH],
                                     start=(kc == 0), stop=(kc == KC - 1))

            def vcopy(c0):
                vscr[0] = vpool.tile([P, TCH], BF16, tag="vscr", name="v_scr")
                nc.vector.tensor_copy(vscr[0][:], ps[0][:])

            def vtr(c0, tt):
                vt = c0 // P + tt
                tr = gpool.tile([P, P], BF16, tag="g", name="v_tr")
                nc.tensor.transpose(tr[:], vscr[0][:, tt * P:(tt + 1) * P],
                                    ident_bf[:])
                nc.vector.tensor_copy(v_comb[:, vt, 0:DK], tr[:, 0:DK])
                nc.vector.tensor_copy(v_comb[:, vt, P:P + DK], tr[:, DK:P])

            yield lambda c0=c0: vstream(c0, 0)
            yield lambda c0=c0: vstream(c0, 4)
            yield lambda c0=c0: vcopy(c0)
            for tt in range(4):
                yield lambda c0=c0, tt=tt: vtr(c0, tt)

    # ---------------- output projection for q-chunk qc ---------------------
    def outproj(qc):
        q0 = qc * TCH
        for m in range(NM):
            ps_y = gpool.tile([P, TCH], F32, tag="g")
            for s in range(NSL):
                nc.tensor.matmul(ps_y[:], wo_sb[:, s, m * P:(m + 1) * P],
                                 oraw[:, s, q0:q0 + TCH],
                                 start=(s == 0), stop=(s == NSL - 1))
            ys = ypool.tile([P, TCH], BF16, tag="ys")
            nc.vector.tensor_copy(ys[:], ps_y[:])
            nc.sync.dma_start(yt[m * P:(m + 1) * P, q0:q0 + TCH], ys[:])

    # ---------------- attention for slice s, interleaved -------------------
    # sched: global iteration counter + deferred PE-side closures.  The
    # normalization's PE work (bc broadcasts, muls, out-proj) is deferred a
    # few iterations into the NEXT q-chunk so the in-order PE stream never
    # parks behind the DVE reciprocal chain.
    sched = {"it": 0, "defer": []}

    def tick_defer():
        while sched["defer"] and sched["defer"][0][0] <= sched["it"]:
            sched["defer"].pop(0)[1]()

    def attention(s, tiles, inter, outproj_here):
        qT, kT, v_comb = tiles
        pend = []
        for qc in range(NCH):
            q0 = qc * TCH
            ps_oA = opool.tile([P, TCH], F32, tag="o", name="ps_oA")
            ps_oB = opool.tile([P, TCH], F32, tag="o", name="ps_oB")

            def attnv(kc, e_t, ps_oA=ps_oA, ps_oB=ps_oB):
                nc.tensor.matmul(ps_oA[:], v_comb[:, kc, 0:P],
                                 e_t[:, 0:TCH],
                                 start=(kc == 0), stop=(kc == NKT - 1))
                nc.tensor.matmul(ps_oB[:], v_comb[:, kc, P:VW],
                                 e_t[:, TCH:2 * TCH],
                                 start=(kc == 0), stop=(kc == NKT - 1))

            for kc in range(NKT):
                kt0 = kc * P
                s_t = spool.tile([P, 2 * TCH], F32, tag="s")
                nc.tensor.matmul(s_t[:, 0:TCH], kT[0:DK, kt0:kt0 + P],
                                 qT[0:DK, q0:q0 + TCH], start=True, stop=True)
                nc.tensor.matmul(s_t[:, TCH:2 * TCH], kT[DK:P, kt0:kt0 + P],
                                 qT[DK:P, q0:q0 + TCH], start=True, stop=True)
                e_t = epool.tile([P, 2 * TCH], BF16, tag="e")
                nc.scalar.activation(e_t[:], s_t[:], EXPF, scale=0.125)
                if len(pend) >= 3:
                    attnv(*pend.pop(0))
                pend.append((kc, e_t))
                sched["it"] += 1
                tick_defer()
                # spread interleaved projection steps evenly over the whole
                # slice so filler work is still available near the q-chunk
                # boundaries
                if inter:
                    steps, done = inter
                    it = qc * NKT + kc
                    want = ((it + 1) * len(steps)) // (NCH * NKT)
                    while inter[1] < min(want, len(steps)):
                        steps[inter[1]]()
                        inter[1] += 1
            while pend:
                attnv(*pend.pop(0))

            # ---- normalization for q-chunk qc ----
            # v_comb halves are [vA | 1 | 0pad] / [vB | 1 | 0pad]: softmax
            # sums land on partition 64 (32-aligned, as DVE PSUM access
            # requires), v outs on partitions 0:64.  The ps_o reads run now
            # (freeing the accumulators); the PE-side tail is deferred.
            nc.vector.tensor_copy(oraw[0:DK, s, q0:q0 + TCH], ps_oA[0:DK, :])
            nc.vector.tensor_copy(oraw[DK:P, s, q0:q0 + TCH], ps_oB[0:DK, :])
            recip_t = sumpool.tile([1, 2 * TCH], F32R, tag="recip")
            nc.vector.reciprocal(recip_t[:, 0:TCH], ps_oA[DK:DK + 1, :])
            nc.vector.reciprocal(recip_t[:, TCH:2 * TCH], ps_oB[DK:DK + 1, :])

            def norm_tail(s=s, q0=q0, recip_t=recip_t):
                bcA = gpool.tile([P, TCH], F32, tag="g", name="bcA")
                nc.tensor.matmul(bcA[0:DK, :], ones_r[:], recip_t[:, 0:TCH],
                                 start=True, stop=True)
                bcB = gpool.tile([P, TCH], F32, tag="g", name="bcB")
                nc.tensor.matmul(bcB[0:DK, :], ones_r[:],
                                 recip_t[:, TCH:2 * TCH],
                                 start=True, stop=True)
                nc.vector.tensor_mul(oraw[0:DK, s, q0:q0 + TCH],
                                     oraw[0:DK, s, q0:q0 + TCH], bcA[0:DK, :])
                nc.vector.tensor_mul(oraw[DK:P, s, q0:q0 + TCH],
                                     oraw[DK:P, s, q0:q0 + TCH], bcB[0:DK, :])

            sched["defer"].append((sched["it"] + 3, norm_tail))
            if outproj_here:
                sched["defer"].append((sched["it"] + 6,
                                       lambda qc=qc: outproj(qc)))

    # ---------------- top-level schedule -----------------------------------
    total = NSL * repeat
    cur = alloc_slice_tiles(0)
    for st in proj_steps(0, cur):
        st()
    for i in range(total):
        s = i % NSL
        inter = []
        nxt = None
        if i + 1 < total:
            nxt = alloc_slice_tiles(i + 1)
            inter = [list(proj_steps((i + 1) % NSL, nxt)), 0]
        attention(s, cur, inter, outproj_here=(s == NSL - 1))
        cur = nxt
    while sched["defer"]:
        sched["defer"].pop(0)[1]()

    for p in reversed(ctxs):
        p.__exit__(None, None, None)


_CACHED = {}


def _get_nc(repeat=1):
    if repeat not in _CACHED:
        _CACHED[repeat] = _build_nc(repeat=repeat)
    return _CACHED[repeat]


def _make_in_maps(x, wq, bq, wk, bk, wv, bv, wo, bo):
    x = np.asarray(x, np.float32)
    wq, bq = np.asarray(wq, np.float32), np.asarray(bq, np.float32)
    wk, bk = np.asarray(wk, np.float32), np.asarray(bk, np.float32)
    wv, bv = np.asarray(wv, np.float32), np.asarray(bv, np.float32)
    wo = np.asarray(wo, np.float32)
    bf = ml_dtypes.bfloat16
    maps = []
    for c in range(NCORES):
        b, h = c // 2, c % 2
        sl = slice(h * NSL * P, (h + 1) * NSL * P)
        maps.append({
            "xt": np.ascontiguousarray(x[b].T).astype(bf),
            "wqm": np.ascontiguousarray(wq[:, sl]).astype(bf),
            "wkm": np.ascontiguousarray(wk[:, sl]).astype(bf),
            "wvm": np.ascontiguousarray(wv[:, sl]).astype(bf),
            "wqb": np.ascontiguousarray(bq[sl]).reshape(NSL, P),
            "wkb": np.ascontiguousarray(bk[sl]).reshape(NSL, P),
            "wo": np.ascontiguousarray(wo[sl, :]).astype(bf),
        })
    return maps


def _gather(results, bo, bv, wo):
    """results: list of 8 dicts with 'yt' [D, S] partial sums.  The device
    skips the v bias; its exact contribution to y is the constant bv @ wo,
    added here along with bo."""
    bias = (np.asarray(bo, np.float64) +
            np.asarray(bv, np.float64) @ np.asarray(wo, np.float64)
            ).astype(np.float32)
    y = np.empty((B, S, D), np.float32)
    for b in range(B):
        yT = results[2 * b]["yt"].astype(np.float32) + \
            results[2 * b + 1]["yt"].astype(np.float32)
        y[b] = yT.T + bias
    return y


def kernel(x, wq, bq, wk, bk, wv, bv, wo, bo):
    nc = _get_nc()
    in_maps = _make_in_maps(x, wq, bq, wk, bk, wv, bv, wo, bo)
    res = run_bass_kernel_spmd(nc, in_maps, core_ids=list(range(NCORES)),
                               trace=False)
    return _gather(res.results, bo, bv, wo)


# revision 8
# speedup vs baseline: 1.4517x; 1.1951x over previous
"""BART attention (B=4, S=2048, D=1024, H=16) on 8 Trainium2 NeuronCores.

Sharding: DP4 x TP2.  Core c owns batch c//2 and head half c%2 (8 heads =
512 projection dims), processed as 4 head-pair "slices" of 128 dims each.
Host sums the two partial y's per batch and adds bo.

Per-core schedule (all matmul inputs bf16; PSUM accumulates f32):
  - x for the core's batch is DMA'd once and stays resident in SBUF.
  - slice s+1's q/k/v projections are interleaved into slice s's attention
    (evenly paced filler steps) so the PE never stalls behind the
    softmax-exp stream on ScalarE; attnv runs 3 iterations behind scores.
  - v streams like q/k then transposes via the PE (fewest PE instructions);
    q/k biases ride the PSUM->SBUF drain (DVE tensor_scalar_add); the v
    bias contribution to y is the constant bv @ wo, added on the host.
  - softmax: exp on ScalarE (fused 1/8 scale); denominators come free as
    ones-columns inside the 128-wide zero-padded (FWL-eligible) v_comb
    stationary operands; per-q-chunk normalization reads the PSUM sum rows
    directly (DVE reciprocal), and its PE work (rank-1 broadcast matmuls +
    multiplies) is deferred into the next q-chunk so the in-order PE
    stream never parks behind the DVE chain.  No DMA round trips.
  - out-proj (contraction over all 4 slices accumulating in PSUM) overlaps
    the last slice's attention; y ships as bf16 partials.
"""
import numpy as np
import ml_dtypes

import concourse.bass as bass
import concourse.mybir as mybir
import concourse.tile as tile
from concourse.bass_utils import run_bass_kernel_spmd
from concourse.masks import make_identity
from concourse.vector_clock import ScopedClock

F32 = mybir.dt.float32
F32R = mybir.dt.float32r
BF16 = mybir.dt.bfloat16
EXPF = mybir.ActivationFunctionType.Exp

B, S, D = 4, 2048, 1024
NCORES = 8
P = 128                        # partitions / head-dims per slice
DK = 64                        # head dim
KC = D // P                    # 8 contraction chunks for projections
TCH = 512                      # token chunk (projection N / q-chunk)
NCH = S // TCH                 # 4 token chunks per batch
NSL = 4                        # head-pair slices per core (4*128 = 512 dims)
NKT = S // P                   # 16 k-tiles per q-chunk
VW = 2 * P                     # 256: [vA | 1 | 0pad][vB | 1 | 0pad] -- each
                               # half is a 128-wide FWL-eligible lhsT
NM = D // P                    # 8 output-dim chunks of out-proj

# ---------------------------------------------------------------------------
# walrus in this toolchain encodes at most ONE sync wait per instruction
# (two on EventSemaphore).  Tile emits more.  Legalize by carrying excess
# waits on same-engine NOPs inserted right before the instruction (engines
# execute in order, so this is equivalent), and by splitting the kernel-tail
# drain's global-clock waits across a chain of drains.
# ---------------------------------------------------------------------------
_split_counter = [0]


def _drop_implied_waits(nc):
    """Drop semaphore waits that are implied by another wait on the same
    instruction: if both sems are sem-ge counters incremented (+1) ONLY by
    one and the same in-order engine, and the update that satisfies wait A
    precedes (or is) the update that satisfies wait B in that engine's
    stream, then B implies A.  Fewer waits -> fewer legalization NOPs."""
    dropped = 0
    for fn in nc.m.functions:
        for bb in fn.blocks:
            pos = {}          # (sem_id, value) -> (engine, seq_in_engine)
            seq = {}          # engine -> instruction counter
            cnt = {}          # sem_id -> cumulative count
            upd_engine = {}   # sem_id -> engine | "MIXED"
            for inst in bb.instructions:
                e = inst.engine
                seq[e] = seq.get(e, 0) + 1
                si = inst.sync_info
                if si is None:
                    continue
                for u in (si.on_update or []):
                    if (u.sync_type != "semaphore"
                            or u.update_mode != "sem-inc"
                            or u.update_value != 1):
                        upd_engine[u.id] = "MIXED"
                        continue
                    if upd_engine.setdefault(u.id, e) != e:
                        upd_engine[u.id] = "MIXED"
                    cnt[u.id] = cnt.get(u.id, 0) + 1
                    pos[(u.id, cnt[u.id])] = (e, seq[e])
            seq2 = {}
            for inst in bb.instructions:
                e = inst.engine
                seq2[e] = seq2.get(e, 0) + 1
                si = inst.sync_info
                if si is None or not si.on_wait:
                    continue
                waits = list(si.on_wait)
                resolved = []
                for w in waits:
                    if (w.sync_type != "semaphore"
                            or w.wait_mode != "sem-ge-imm"
                            or upd_engine.get(w.id) in (None, "MIXED")):
                        resolved.append(None)
                    else:
                        resolved.append(pos.get((w.id, w.wait_value)))
                # rule 1: a wait satisfied by an EARLIER instruction on this
                # very engine is implied by in-order execution
                for i, r in enumerate(resolved):
                    if r is not None and r[0] == e and r[1] < seq2[e]:
                        resolved[i] = "DROP"
                # rule 2: among remaining resolved waits sharing an updater
                # engine, only the latest-satisfied one is needed
                best = {}
                for i, r in enumerate(resolved):
                    if r is None or r == "DROP":
                        continue
                    if r[0] not in best:
                        best[r[0]] = i
                    elif resolved[best[r[0]]][1] < r[1]:
                        resolved[best[r[0]]] = "DROP"
                        best[r[0]] = i
                    else:
                        resolved[i] = "DROP"
                keep = [w for w, r in zip(waits, resolved) if r != "DROP"]
                dropped += len(waits) - len(keep)
                if len(keep) < len(waits):
                    si.on_wait = keep
    return dropped


def _legalize_waits(nc):
    inserted = 0
    for fn in nc.m.functions:
        for bb in fn.blocks:
            new_insts = []
            changed = False
            for inst in bb.instructions:
                si = inst.sync_info
                waits = list(si.on_wait) if si is not None and si.on_wait else []
                cap = 2 if inst.opcode == "EventSemaphore" else 1
                if len(waits) > cap:
                    excess, keep = waits[:-cap], waits[-cap:]
                    for w in excess:
                        _split_counter[0] += 1
                        nop = mybir.InstNoOp(
                            name=f"I-waitsplit-{_split_counter[0]}", ins=[], outs=[]
                        )
                        nop.engine = inst.engine
                        nop.sync_info = mybir.SyncInfo(on_wait=[w], on_update=[])
                        new_insts.append(nop)
                        inserted += 1
                    si.on_wait = keep
                    changed = True
                new_insts.append(inst)
            if changed:
                bb.instructions.clear()
                for i in new_insts:
                    bb.instructions.append(i)
    return inserted


class _TC(tile.TileContext):
    def _drain_and_barrier(self, tick_clock, wait_clock):
        drain_inst = self.nc.sync.drain()
        wait_clock.add_sem_waits(
            drain_inst.ins, ScopedClock({None: tick_clock.global_clock})
        )
        si = drain_inst.ins.sync_info
        waits = list(si.on_wait or []) if si is not None else []
        if len(waits) > 1:
            si.on_wait = [waits[0]]
            for w in waits[1:]:
                d = self.nc.sync.drain()
                dsi = d.ins.sync_info
                if dsi is None:
                    d.ins.sync_info = mybir.SyncInfo(on_wait=[w], on_update=[])
                else:
                    dsi.on_wait = [w]
        self.nc.all_engine_barrier()
        assert self.sems is not None
        popped = self.nc._tile_sem_poison_stack.pop()
        assert popped is self._sem_poison
        self.nc.clear_and_free_semaphores(list(self.sems.allocated().values()))
        self.nc.all_engine_barrier()


# ---------------------------------------------------------------------------
# device program (identical on all 8 cores; only input data differs)
# ---------------------------------------------------------------------------
def _build_nc(repeat=1):
    nc = bass.Bass("TRN2", target_bir_lowering=False, debug=False,
                   num_devices=NCORES)
    xt = nc.dram_tensor("xt", [D, S], BF16, kind="ExternalInput").ap()
    wqm = nc.dram_tensor("wqm", [D, NSL * P], BF16, kind="ExternalInput").ap()
    wkm = nc.dram_tensor("wkm", [D, NSL * P], BF16, kind="ExternalInput").ap()
    wvm = nc.dram_tensor("wvm", [D, NSL * P], BF16, kind="ExternalInput").ap()
    wqb = nc.dram_tensor("wqb", [NSL, P], F32, kind="ExternalInput").ap()
    wkb = nc.dram_tensor("wkb", [NSL, P], F32, kind="ExternalInput").ap()
    wot = nc.dram_tensor("wo", [NSL * P, D], BF16, kind="ExternalInput").ap()
    yt = nc.dram_tensor("yt", [D, S], BF16, kind="ExternalOutput").ap()

    with _TC(nc) as tc, nc.allow_low_precision(
            reason="bf16 matmul inputs; 2e-2 harness tolerance"):
        _emit(nc, tc, xt, wqm, wkm, wvm, wqb, wkb, wot, yt, repeat=repeat)
    _drop_implied_waits(nc)
    _legalize_waits(nc)
    return nc


def _emit(nc, tc, xt, wqm, wkm, wvm, wqb, wkb, wot, yt, repeat=1):
    ctxs = []

    def pool(name, bufs, space="SBUF"):
        p = tc.tile_pool(name=name, bufs=bufs, space=space)
        ctxs.append(p)
        return p.__enter__()

    wpool = pool("w", 1)
    persist = pool("persist", 1)
    qkpool = pool("qk", 2)
    vpool = pool("v", 2)
    epool = pool("e", 4)
    sumpool = pool("sums", 2)
    ypool = pool("yst", 2)
    spool = pool("ps_s", 2, space="PSUM")      # [128,1024] f32 = 2 banks/slot
    opool = pool("ps_o", 2, space="PSUM")      # [65,512] 1 bank/slot (A+B)
    gpool = pool("ps_g", 2, space="PSUM")      # [128,512] 1 bank/slot (shared)

    # ---- constants / weights / resident x ----
    # DMA queue order is issue order: x chunk 0 and wq first so the first
    # projection matmuls start as early as possible.
    x_res = persist.tile([P, KC, S], BF16)      # resident x [d%128, d//128, t]
    wq_sb = wpool.tile([P, KC, NSL * P], BF16)
    wk_sb = wpool.tile([P, KC, NSL * P], BF16)
    wv_sb = wpool.tile([P, KC, NSL * P], BF16)
    wo_sb = wpool.tile([P, NSL, D], BF16)
    bq_sb = wpool.tile([P, NSL], F32)
    bk_sb = wpool.tile([P, NSL], F32)

    def xload(c0):
        nc.sync.dma_start(
            x_res[:, :, c0:c0 + TCH],
            xt[:, c0:c0 + TCH].rearrange("(k p) n -> p k n", p=P))

    xload(0)
    wqr = wqm.rearrange("(k p) d -> p k d", p=P)
    wkr = wkm.rearrange("(k p) d -> p k d", p=P)
    wvr = wvm.rearrange("(k p) d -> p k d", p=P)
    nc.sync.dma_start(wq_sb[:, :, 0:P], wqr[:, :, 0:P])
    nc.sync.dma_start(bq_sb[:], wqb.rearrange("s p -> p s"))
    nc.sync.dma_start(wk_sb[:, :, 0:P], wkr[:, :, 0:P])
    nc.sync.dma_start(bk_sb[:], wkb.rearrange("s p -> p s"))
    nc.sync.dma_start(wv_sb[:, :, 0:P], wvr[:, :, 0:P])
    for c in range(1, NCH):
        xload(c * TCH)
    nc.sync.dma_start(wq_sb[:, :, P:NSL * P], wqr[:, :, P:NSL * P])
    nc.sync.dma_start(wk_sb[:, :, P:NSL * P], wkr[:, :, P:NSL * P])
    nc.sync.dma_start(wv_sb[:, :, P:NSL * P], wvr[:, :, P:NSL * P])
    nc.sync.dma_start(wo_sb[:], wot.rearrange("(s p) d -> p s d", p=P))

    ones_f32 = wpool.tile([P, TCH], F32)
    nc.vector.memset(ones_f32[:], 1.0)
    ones_r = wpool.tile([1, DK], F32R)
    nc.vector.tensor_copy(ones_r[:], ones_f32[0:1, 0:DK])
    ident_f32 = wpool.tile([P, P], F32)
    make_identity(nc, ident_f32[:])
    ident_bf = wpool.tile([P, P], BF16)
    nc.vector.tensor_copy(ident_bf[:], ident_f32[:])

    # all slices' normalized attention outputs: [dim%128, slice, tok]
    oraw = persist.tile([P, NSL, S], BF16)

    # v_comb ping-pong pair is persistent: the zero pad + ones columns are
    # written once here, outside the repeat loop; vdrain only overwrites the
    # vA/vB column blocks.
    v_combs = []
    for pp in range(2):
        v_c = persist.tile([P, NKT, VW], BF16, name=f"v_comb{pp}")
        nc.vector.memset(v_c[:], 0.0)
        for half in range(2):
            o = half * P + DK
            nc.vector.tensor_copy(
                v_c[:, :, o:o + 1], ones_f32[:, 0:1].broadcast_to([P, NKT, 1]))
        v_combs.append(v_c)

    def alloc_slice_tiles(i):
        qT = qkpool.tile([P, S], BF16, tag="qT")
        kT = qkpool.tile([P, S], BF16, tag="kT")
        return qT, kT, v_combs[i % 2]

    # ---------------- projection steps for slice s (generator) -------------
    def proj_steps(s, tiles):
        """Yields closures; each emits a small group of instructions that
        computes slice s's qT/kT/v_comb into `tiles`."""
        qT, kT, v_comb = tiles
        ps = [None]
        for c in range(NCH):
            c0 = c * TCH

            def qkmm(c0, w_sb, lo):
                if lo == 0:
                    ps[0] = gpool.tile([P, TCH], F32, tag="g", name="qk_ps")
                for kc in range(lo, lo + 4):
                    nc.tensor.matmul(ps[0][:], w_sb[:, kc, s * P:(s + 1) * P],
                                     x_res[:, kc, c0:c0 + TCH],
                                     start=(kc == 0), stop=(kc == KC - 1))

            def qkdrain(c0, dst, b_sb):
                nc.vector.tensor_scalar_add(dst[:, c0:c0 + TCH], ps[0][:],
                                            b_sb[:, s:s + 1])

            yield lambda c0=c0: qkmm(c0, wq_sb, 0)
            yield lambda c0=c0: qkmm(c0, wq_sb, 4)
            yield lambda c0=c0: qkdrain(c0, qT, bq_sb)
            yield lambda c0=c0: qkmm(c0, wk_sb, 0)
            yield lambda c0=c0: qkmm(c0, wk_sb, 4)
            yield lambda c0=c0: qkdrain(c0, kT, bk_sb)

            # v streamed like q/k ([head-dim, tok], N=512 matmuls — far fewer
            # PE instructions than token-major tiles), then transposed into
            # v_comb via the PE with an identity.  bv is NOT added on device:
            # normalized attnv output with biased v is (out + bv), and bv's
            # contribution to y is the constant bv @ wo, which the host adds
            # exactly.
            vscr = [None]

            def vstream(c0, lo):
                if lo == 0:
                    ps[0] = gpool.tile([P, TCH], F32, tag="g", name="v_ps")
                for kc in range(lo, lo + 4):
                    nc.tensor.matmul(ps[0][:], wv_sb[:, kc, s * P:(s + 1) * P],
                                     x_res[:, kc, c0:c0 + TCH],
                                     start=(kc == 0), stop=(kc == KC - 1))

            def vcopy(c0):
                vscr[0] = vpool.tile([P, TCH], BF16, tag="vscr", name="v_scr")
                nc.vector.tensor_copy(vscr[0][:], ps[0][:])

            def vtr(c0, tt):
                vt = c0 // P + tt
                tr = gpool.tile([P, P], BF16, tag="g", name="v_tr")
                nc.tensor.transpose(tr[:], vscr[0][:, tt * P:(tt + 1) * P],
                                    ident_bf[:])
                nc.vector.tensor_copy(v_comb[:, vt, 0:DK], tr[:, 0:DK])
                nc.vector.tensor_copy(v_comb[:, vt, P:P + DK], tr[:, DK:P])

            yield lambda c0=c0: vstream(c0, 0)
            yield lambda c0=c0: vstream(c0, 4)
            yield lambda c0=c0: vcopy(c0)
            for tt in range(4):
                yield lambda c0=c0, tt=tt: vtr(c0, tt)

    # ---------------- output projection for q-chunk qc ---------------------
    def outproj(qc):
        q0 = qc * TCH
        for m in range(NM):
            ps_y = gpool.tile([P, TCH], F32, tag="g")
            for s in range(NSL):
                nc.tensor.matmul(ps_y[:], wo_sb[:, s, m * P:(m + 1) * P],
                                 oraw[:, s, q0:q0 + TCH],
                                 start=(s == 0), stop=(s == NSL - 1))
            ys = ypool.tile([P, TCH], BF16, tag="ys")
            nc.vector.tensor_copy(ys[:], ps_y[:])
            nc.sync.dma_start(yt[m * P:(m + 1) * P, q0:q0 + TCH], ys[:])

    # ---------------- attention for slice s, interleaved -------------------
    # sched: global iteration counter + deferred PE-side closures.  The
    # normalization's PE work (bc broadcasts, muls, out-proj) is deferred a
    # few iterations into the NEXT q-chunk so the in-order PE stream never
    # parks behind the DVE reciprocal chain.
    sched = {"it": 0, "defer": []}

    def tick_defer():
        while sched["defer"] and sched["defer"][0][0] <= sched["it"]:
            sched["defer"].pop(0)[1]()

    def attention(s, tiles, inter, outproj_here):
        qT, kT, v_comb = tiles
        pend = []
        for qc in range(NCH):
            q0 = qc * TCH
            ps_oA = opool.tile([P, TCH], F32, tag="o", name="ps_oA")
            ps_oB = opool.tile([P, TCH], F32, tag="o", name="ps_oB")

            def attnv(kc, e_t, ps_oA=ps_oA, ps_oB=ps_oB):
                nc.tensor.matmul(ps_oA[:], v_comb[:, kc, 0:P],
                                 e_t[:, 0:TCH],
                                 start=(kc == 0), stop=(kc == NKT - 1))
                nc.tensor.matmul(ps_oB[:], v_comb[:, kc, P:VW],
                                 e_t[:, TCH:2 * TCH],
                                 start=(kc == 0), stop=(kc == NKT - 1))

            for kc in range(NKT):
                kt0 = kc * P
                s_t = spool.tile([P, 2 * TCH], F32, tag="s")
                nc.tensor.matmul(s_t[:, 0:TCH], kT[0:DK, kt0:kt0 + P],
                                 qT[0:DK, q0:q0 + TCH], start=True, stop=True)
                nc.tensor.matmul(s_t[:, TCH:2 * TCH], kT[DK:P, kt0:kt0 + P],
                                 qT[DK:P, q0:q0 + TCH], start=True, stop=True)
                e_t = epool.tile([P, 2 * TCH], BF16, tag="e")
                nc.scalar.activation(e_t[:], s_t[:], EXPF, scale=0.125)
                if len(pend) >= 3:
                    attnv(*pend.pop(0))
                pend.append((kc, e_t))
                sched["it"] += 1
                tick_defer()
                # spread interleaved projection steps evenly over the whole
                # slice so filler work is still available near the q-chunk
                # boundaries
                if inter:
                    steps, done = inter
                    it = qc * NKT + kc
                    want = ((it + 1) * len(steps)) // (NCH * NKT)
                    while inter[1] < min(want, len(steps)):
                        steps[inter[1]]()
                        inter[1] += 1
            while pend:
                attnv(*pend.pop(0))

            # ---- normalization for q-chunk qc ----
            # v_comb halves are [vA | 1 | 0pad] / [vB | 1 | 0pad]: softmax
            # sums land on partition 64 (32-aligned, as DVE PSUM access
            # requires), v outs on partitions 0:64.  The ps_o reads run now
            # (freeing the accumulators); the PE-side tail is deferred.
            nc.vector.tensor_copy(oraw[0:DK, s, q0:q0 + TCH], ps_oA[0:DK, :])
            nc.vector.tensor_copy(oraw[DK:P, s, q0:q0 + TCH], ps_oB[0:DK, :])
            recip_t = sumpool.tile([1, 2 * TCH], F32R, tag="recip")
            nc.vector.reciprocal(recip_t[:, 0:TCH], ps_oA[DK:DK + 1, :])
            nc.vector.reciprocal(recip_t[:, TCH:2 * TCH], ps_oB[DK:DK + 1, :])

            def norm_tail(s=s, q0=q0, recip_t=recip_t):
                bcA = gpool.tile([P, TCH], F32, tag="g", name="bcA")
                nc.tensor.matmul(bcA[0:DK, :], ones_r[:], recip_t[:, 0:TCH],
                                 start=True, stop=True)
                bcB = gpool.tile([P, TCH], F32, tag="g", name="bcB")
                nc.tensor.matmul(bcB[0:DK, :], ones_r[:],
                                 recip_t[:, TCH:2 * TCH],
                                 start=True, stop=True)
                nc.vector.tensor_mul(oraw[0:DK, s, q0:q0 + TCH],
                                     oraw[0:DK, s, q0:q0 + TCH], bcA[0:DK, :])
                nc.vector.tensor_mul(oraw[DK:P, s, q0:q0 + TCH],
                                     oraw[DK:P, s, q0:q0 + TCH], bcB[0:DK, :])

            sched["defer"].append((sched["it"] + 3, norm_tail))
            if outproj_here:
                sched["defer"].append((sched["it"] + 6,
                                       lambda qc=qc: outproj(qc)))

    # ---------------- top-level schedule -----------------------------------
    total = NSL * repeat
    cur = alloc_slice_tiles(0)
    for st in proj_steps(0, cur):
        st()
    for i in range(total):
        s = i % NSL
        inter = []
        nxt = None
        if i + 1 < total:
            nxt = alloc_slice_tiles(i + 1)
            inter = [list(proj_steps((i + 1) % NSL, nxt)), 0]
        attention(s, cur, inter, outproj_here=(s == NSL - 1))
        cur = nxt
    while sched["defer"]:
        sched["defer"].pop(0)[1]()

    for p in reversed(ctxs):
        p.__exit__(None, None, None)


_CACHED = {}


def _get_nc(repeat=1):
    if repeat not in _CACHED:
        _CACHED[repeat] = _build_nc(repeat=repeat)
    return _CACHED[repeat]


def _make_in_maps(x, wq, bq, wk, bk, wv, bv, wo, bo):
    x = np.asarray(x, np.float32)
    wq, bq = np.asarray(wq, np.float32), np.asarray(bq, np.float32)
    wk, bk = np.asarray(wk, np.float32), np.asarray(bk, np.float32)
    wv, bv = np.asarray(wv, np.float32), np.asarray(bv, np.float32)
    wo = np.asarray(wo, np.float32)
    bf = ml_dtypes.bfloat16
    maps = []
    for c in range(NCORES):
        b, h = c // 2, c % 2
        sl = slice(h * NSL * P, (h + 1) * NSL * P)
        maps.append({
            "xt": np.ascontiguousarray(x[b].T).astype(bf),
            "wqm": np.ascontiguousarray(wq[:, sl]).astype(bf),
            "wkm": np.ascontiguousarray(wk[:, sl]).astype(bf),
            "wvm": np.ascontiguousarray(wv[:, sl]).astype(bf),
            "wqb": np.ascontiguousarray(bq[sl]).reshape(NSL, P),
            "wkb": np.ascontiguousarray(bk[sl]).reshape(NSL, P),
            "wo": np.ascontiguousarray(wo[sl, :]).astype(bf),
        })
    return maps


def _gather(results, bo, bv, wo):
    """results: list of 8 dicts with 'yt' [D, S] partial sums.  The device
    skips the v bias; its exact contribution to y is the constant bv @ wo,
    added here along with bo."""
    bias = (np.asarray(bo, np.float64) +
            np.asarray(bv, np.float64) @ np.asarray(wo, np.float64)
            ).astype(np.float32)
    y = np.empty((B, S, D), np.float32)
    for b in range(B):
        yT = results[2 * b]["yt"].astype(np.float32) + \
            results[2 * b + 1]["yt"].astype(np.float32)
        y[b] = yT.T + bias
    return y


def kernel(x, wq, bq, wk, bk, wv, bv, wo, bo):
    nc = _get_nc()
    in_maps = _make_in_maps(x, wq, bq, wk, bk, wv, bv, wo, bo)
    res = run_bass_kernel_spmd(nc, in_maps, core_ids=list(range(NCORES)),
                               trace=False)
    return _gather(res.results, bo, bv, wo)
